# revision 46
# baseline (speedup 1.0000x reference)
"""CaptionEmbedder kernel for Trainium2 (Bass), 8-core data-parallel.

Semantics (matching the reference):
    ent_idx  = clamp-to-49 of (caption_indices - 32000)   (oob -> 49)
    word_idx = caption_indices if < 32000 else pad_token
    out[b,l] = entities_encoded[b, ent_idx]  if caption_masks[b,l,0] == 1
               else word_embedding[word_idx]

Strategy: shard the batch dim (8 batches/core). The host concatenates the
core's entity shard [400, 512] onto the word table -> one combined bf16
table [32400, 512] per core (rel err <= 2^-9, far under the 2e-2 gate),
and computes the final combined row index per token on the host:
  combined_row = mask ? (32000 + 50*local_b + clamped_ent) : word_idx
The device is a pure streaming gather. The pacing stream is Q7 SWDGE
descriptor generation (~8.7ns/row measured; only Q7 cores 0-1 can
address all partitions, so it cannot be parallelized). Everything else
stays off that path and off the shared SDMA engines: per-column native
indirect DMAs gather bf16 rows and cast to f32 in the DMA itself (SWDGE
supports dtype casts), halving gather-side HBM reads; f32 stores go as
7 two-column chunks on a single HWDGE queue (sync) so the SDMA packet
round-robin gives the gather queue a full half share and completions
track the gen stream.

Token layout: token t lives at SBUF [t%128, t//128]; the host packs the
index array in that order and transposes the output back.
"""

import os
import sys
from functools import lru_cache

import numpy as np

for _p in ("/opt/trn_rl_repo",):
    if _p not in sys.path:
        sys.path.insert(0, _p)

# Problem shapes (hardcoded per contest contract).
V = 32000          # vocab size
B = 64             # batch
L = 200            # caption length
N_ENT = 50         # entities per batch
D = 512            # embedding dim
N_CORES = 8
B_LOC = B // N_CORES            # 8 batches per core
TOK = B_LOC * L                 # 1600 tokens per core
P = 128                         # SBUF partitions
COLS = -(-TOK // P)             # 13 columns of 128 tokens
TOK_PAD = P * COLS              # 1664
TBL = V + B_LOC * N_ENT         # 32400 rows in combined table


@lru_cache(maxsize=2)
def _build():
    import concourse.bacc as bacc
    import concourse.bass as bass
    from concourse import mybir

    i32 = mybir.dt.int32
    f32 = mybir.dt.float32
    bf16 = mybir.dt.bfloat16

    nc = bacc.Bacc(
        "TRN2", target_bir_lowering=False, debug=False, num_swdge_queues=2
    )

    tbl_h = nc.dram_tensor("table", [TBL, D], bf16, kind="ExternalInput")
    comb_h = nc.dram_tensor("comb", [P, COLS], i32, kind="ExternalInput")
    out_h = nc.dram_tensor("out", [P, COLS, D], f32, kind="ExternalOutput")
    tbl_ap = tbl_h.ap()
    out_ap = out_h.ap()

    comb_sb = nc.alloc_sbuf_tensor("comb_sb", [P, COLS], i32).ap()
    emb_f = nc.alloc_sbuf_tensor("emb_f", [P, COLS, D], f32).ap()

    sem_ld = nc.alloc_semaphore("sem_ld")
    sem_ld2 = nc.alloc_semaphore("sem_ld2")
    sem_gs = [nc.alloc_semaphore(f"sem_g{c}") for c in range(COLS)]
    sem_s = nc.alloc_semaphore("sem_s")

    def vp(c):  # valid partitions in column c (64 on the last column)
        return min(P, TOK - c * P)

    with nc.Block() as block:

        @block.sync
        def _(sync):
            # index load via HWDGE as sync's first instructions, split so
            # the first gather columns can start before the full load lands
            sync.dma_start(
                out=comb_sb[:, 0:2], in_=comb_h.ap()[:, 0:2]
            ).then_inc(sem_ld, 16)
            sync.dma_start(
                out=comb_sb[:, 2:COLS], in_=comb_h.ap()[:, 2:COLS]
            ).then_inc(sem_ld2, 16)
            # stores: 2-col chunks on a single HWDGE queue; completion is
            # covered by the runtime's end-of-execution DMA quiesce (no
            # final sem wait on the engine program)
            for c0 in range(0, COLS, 2):
                cw = min(2, COLS - c0)
                for c in range(c0, c0 + cw):
                    sync.wait_ge(sem_gs[c], 16)
                ce = c0 + cw - 1  # last col of chunk (may be short: 64)
                if vp(ce) == P:
                    sync.dma_start(
                        out=out_ap[:, c0 : c0 + cw, :],
                        in_=emb_f[:, c0 : c0 + cw, :],
                    ).then_inc(sem_s, 16)
                else:
                    if cw > 1:
                        sync.dma_start(
                            out=out_ap[:, c0:ce, :], in_=emb_f[:, c0:ce, :]
                        ).then_inc(sem_s, 16)
                    sync.dma_start(
                        out=out_ap[0 : vp(ce), ce : ce + 1, :],
                        in_=emb_f[0 : vp(ce), ce : ce + 1, :],
                    ).then_inc(sem_s, 16)

        @block.gpsimd
        def _(gpsimd):
            gpsimd.wait_ge(sem_ld, 16)
            for c in range(COLS):
                if c == 2:
                    gpsimd.wait_ge(sem_ld2, 16)
                # bf16 rows cast to f32 by the DMA itself (SWDGE casts);
                # columns alternate across the two SWDGE queues so each
                # queue carries half the backlog and the pair outvotes the
                # store queue in the SDMA packet round-robin
                bi = gpsimd.indirect_dma_start(
                    out=emb_f[0 : vp(c), c, :],
                    out_offset=None,
                    in_=tbl_ap[:, :],
                    in_offset=bass.IndirectOffsetOnAxis(
                        ap=comb_sb[0 : vp(c), c : c + 1], axis=0
                    ),
                )
                if c % 2:
                    bi.ins.queue = "qPoolDynamic1"
                bi.then_inc(sem_gs[c], 16)

    # Block exit emitted an all-engine barrier; reset our semaphores so the
    # NEFF is re-executable.
    for s in (sem_ld, sem_ld2, *sem_gs, sem_s):
        nc.gpsimd.sem_clear(s)

    nc.compile()
    return nc


def _wrap(a: np.ndarray) -> np.ndarray:
    """Token t -> [t%128, t//128]."""
    return np.ascontiguousarray(a.reshape(COLS, P).T)


def _shard_inputs(caption_indices, entities_encoded, word_embedding,
                  pad_val, caption_masks):
    import ml_dtypes

    bf16 = ml_dtypes.bfloat16
    caption_indices = np.asarray(caption_indices, dtype=np.int64)
    caption_masks = np.asarray(caption_masks, dtype=np.int64).reshape(B, L)
    entities_bf = np.asarray(entities_encoded).astype(bf16)
    word_bf = np.asarray(word_embedding).astype(bf16)

    # combined row index per token (computed on host; the device is a pure
    # streaming gather)
    ent = caption_indices - V
    ent = np.where((ent < 0) | (ent >= N_ENT), N_ENT - 1, ent)
    word = np.where(caption_indices >= V, pad_val, caption_indices)
    b_loc = (np.arange(B) % B_LOC)[:, None]
    comb = np.where(
        caption_masks == 1, V + N_ENT * b_loc + ent, word
    ).astype(np.int32)

    in_maps = []
    for i in range(N_CORES):
        sl = slice(i * B_LOC, (i + 1) * B_LOC)
        tbl = np.concatenate(
            [word_bf, entities_bf[sl].reshape(B_LOC * N_ENT, D)], axis=0
        )
        comb_pad = np.zeros(TOK_PAD, dtype=np.int32)  # pad -> row 0, harmless
        comb_pad[:TOK] = comb[sl].reshape(-1)
        in_maps.append(
            {"table": np.ascontiguousarray(tbl), "comb": _wrap(comb_pad)}
        )
    return in_maps


LAST_RESULTS = None  # BassKernelResults of the most recent run (for test.py)


def kernel(caption_indices, entities_encoded, word_embedding, pad_token,
           caption_masks):
    global LAST_RESULTS
    from concourse.bass_utils import run_bass_kernel_spmd

    nc = _build()
    in_maps = _shard_inputs(caption_indices, entities_encoded,
                            word_embedding, int(pad_token), caption_masks)
    res = run_bass_kernel_spmd(
        nc,
        in_maps,
        list(range(N_CORES)),
        trace=bool(os.environ.get("CAPEMB_TRACE")),
    )
    LAST_RESULTS = res
    out = np.empty((B, L, D), dtype=np.float32)
    for i in range(N_CORES):
        toks = np.transpose(res.results[i]["out"], (1, 0, 2)).reshape(
            TOK_PAD, D
        )[:TOK]
        out[i * B_LOC : (i + 1) * B_LOC] = toks.reshape(B_LOC, L, D)
    return out


# revision 48
# speedup vs baseline: 1.0350x; 1.0350x over previous
"""CaptionEmbedder kernel for Trainium2 (Bass), 8-core data-parallel.

Semantics (matching the reference):
    ent_idx  = clamp-to-49 of (caption_indices - 32000)   (oob -> 49)
    word_idx = caption_indices if < 32000 else pad_token
    out[b,l] = entities_encoded[b, ent_idx]  if caption_masks[b,l,0] == 1
               else word_embedding[word_idx]

Strategy: shard the batch dim (8 batches/core). The host concatenates the
core's entity shard [400, 512] onto the word table -> one combined bf16
table [32400, 512] per core (rel err <= 2^-9, far under the 2e-2 gate),
and computes the final combined row index per token on the host:
  combined_row = mask ? (32000 + 50*local_b + clamped_ent) : word_idx
The device is a pure streaming gather. The pacing stream is Q7 SWDGE
descriptor generation (~8.7ns/row measured; only Q7 cores 0-1 can
address all partitions, so it cannot be parallelized). Everything else
stays off that path and off the shared SDMA engines: per-column native
indirect DMAs gather bf16 rows and cast to f32 in the DMA itself (SWDGE
supports dtype casts), halving gather-side HBM reads; f32 stores go as
7 two-column chunks on a single HWDGE queue (sync) so the SDMA packet
round-robin gives the gather queue a full half share and completions
track the gen stream.

Token layout: token t lives at SBUF [t%128, t//128]; the host packs the
index array in that order and transposes the output back.
"""

import os
import sys
from functools import lru_cache

import numpy as np

for _p in ("/opt/trn_rl_repo",):
    if _p not in sys.path:
        sys.path.insert(0, _p)

# Problem shapes (hardcoded per contest contract).
V = 32000          # vocab size
B = 64             # batch
L = 200            # caption length
N_ENT = 50         # entities per batch
D = 512            # embedding dim
N_CORES = 8
B_LOC = B // N_CORES            # 8 batches per core
TOK = B_LOC * L                 # 1600 tokens per core
P = 128                         # SBUF partitions
COLS = -(-TOK // P)             # 13 columns of 128 tokens
TOK_PAD = P * COLS              # 1664
TBL = V + B_LOC * N_ENT         # 32400 rows in combined table


@lru_cache(maxsize=2)
def _build():
    import concourse.bacc as bacc
    import concourse.bass as bass
    from concourse import mybir

    i32 = mybir.dt.int32
    f32 = mybir.dt.float32
    bf16 = mybir.dt.bfloat16

    nc = bacc.Bacc(
        "TRN2", target_bir_lowering=False, debug=False, num_swdge_queues=4
    )

    tbl_h = nc.dram_tensor("table", [TBL, D], bf16, kind="ExternalInput")
    comb_h = nc.dram_tensor("comb", [P, COLS], i32, kind="ExternalInput")
    out_h = nc.dram_tensor("out", [P, COLS, D], f32, kind="ExternalOutput")
    tbl_ap = tbl_h.ap()
    out_ap = out_h.ap()

    comb_sb = nc.alloc_sbuf_tensor("comb_sb", [P, COLS], i32).ap()
    emb_f = nc.alloc_sbuf_tensor("emb_f", [P, COLS, D], f32).ap()

    sem_ld = nc.alloc_semaphore("sem_ld")
    sem_ld2 = nc.alloc_semaphore("sem_ld2")
    sem_gs = [nc.alloc_semaphore(f"sem_g{c}") for c in range(COLS)]
    sem_s = nc.alloc_semaphore("sem_s")

    def vp(c):  # valid partitions in column c (64 on the last column)
        return min(P, TOK - c * P)

    with nc.Block() as block:

        @block.sync
        def _(sync):
            # index load via HWDGE as sync's first instructions, split so
            # the first gather columns can start before the full load lands
            sync.dma_start(
                out=comb_sb[:, 0:2], in_=comb_h.ap()[:, 0:2]
            ).then_inc(sem_ld, 16)
            sync.dma_start(
                out=comb_sb[:, 2:COLS], in_=comb_h.ap()[:, 2:COLS]
            ).then_inc(sem_ld2, 16)
            # stores: 2-col chunks on a single HWDGE queue; completion is
            # covered by the runtime's end-of-execution DMA quiesce (no
            # final sem wait). Col 12's gather is generated before 10/11's
            # (see gpsimd order), so its small store clears first and the
            # program ends on the (10,11) store.
            for c0 in range(0, 10, 2):
                sync.wait_ge(sem_gs[c0], 16)
                sync.wait_ge(sem_gs[c0 + 1], 16)
                sync.dma_start(
                    out=out_ap[:, c0 : c0 + 2, :],
                    in_=emb_f[:, c0 : c0 + 2, :],
                ).then_inc(sem_s, 16)
            sync.wait_ge(sem_gs[12], 16)
            sync.dma_start(
                out=out_ap[0 : vp(12), 12:13, :],
                in_=emb_f[0 : vp(12), 12:13, :],
            ).then_inc(sem_s, 16)
            sync.wait_ge(sem_gs[10], 16)
            sync.wait_ge(sem_gs[11], 16)
            sync.dma_start(
                out=out_ap[:, 10:12, :], in_=emb_f[:, 10:12, :]
            ).then_inc(sem_s, 16)

        @block.gpsimd
        def _(gpsimd):
            gpsimd.wait_ge(sem_ld, 16)
            # col 12 generated before 10/11 so its store isn't the one
            # gated by the stream's final gather
            order = list(range(10)) + [12, 10, 11]
            for k, c in enumerate(order):
                if k == 2:
                    gpsimd.wait_ge(sem_ld2, 16)
                # bf16 rows cast to f32 by the DMA itself (SWDGE casts);
                # columns rotate across all four SWDGE queues so each
                # carries a quarter of the backlog and together they take
                # 4/5 of the SDMA packet round-robin vs the store queue
                bi = gpsimd.indirect_dma_start(
                    out=emb_f[0 : vp(c), c, :],
                    out_offset=None,
                    in_=tbl_ap[:, :],
                    in_offset=bass.IndirectOffsetOnAxis(
                        ap=comb_sb[0 : vp(c), c : c + 1], axis=0
                    ),
                )
                if k % 4:
                    bi.ins.queue = f"qPoolDynamic{k % 4}"
                bi.then_inc(sem_gs[c], 16)

    # Block exit emitted an all-engine barrier; reset our semaphores so the
    # NEFF is re-executable.
    for s in (sem_ld, sem_ld2, *sem_gs, sem_s):
        nc.gpsimd.sem_clear(s)

    nc.compile()
    return nc


def _wrap(a: np.ndarray) -> np.ndarray:
    """Token t -> [t%128, t//128]."""
    return np.ascontiguousarray(a.reshape(COLS, P).T)


def _shard_inputs(caption_indices, entities_encoded, word_embedding,
                  pad_val, caption_masks):
    import ml_dtypes

    bf16 = ml_dtypes.bfloat16
    caption_indices = np.asarray(caption_indices, dtype=np.int64)
    caption_masks = np.asarray(caption_masks, dtype=np.int64).reshape(B, L)
    entities_bf = np.asarray(entities_encoded).astype(bf16)
    word_bf = np.asarray(word_embedding).astype(bf16)

    # combined row index per token (computed on host; the device is a pure
    # streaming gather)
    ent = caption_indices - V
    ent = np.where((ent < 0) | (ent >= N_ENT), N_ENT - 1, ent)
    word = np.where(caption_indices >= V, pad_val, caption_indices)
    b_loc = (np.arange(B) % B_LOC)[:, None]
    comb = np.where(
        caption_masks == 1, V + N_ENT * b_loc + ent, word
    ).astype(np.int32)

    in_maps = []
    for i in range(N_CORES):
        sl = slice(i * B_LOC, (i + 1) * B_LOC)
        tbl = np.concatenate(
            [word_bf, entities_bf[sl].reshape(B_LOC * N_ENT, D)], axis=0
        )
        comb_pad = np.zeros(TOK_PAD, dtype=np.int32)  # pad -> row 0, harmless
        comb_pad[:TOK] = comb[sl].reshape(-1)
        in_maps.append(
            {"table": np.ascontiguousarray(tbl), "comb": _wrap(comb_pad)}
        )
    return in_maps


LAST_RESULTS = None  # BassKernelResults of the most recent run (for test.py)


def kernel(caption_indices, entities_encoded, word_embedding, pad_token,
           caption_masks):
    global LAST_RESULTS
    from concourse.bass_utils import run_bass_kernel_spmd

    nc = _build()
    in_maps = _shard_inputs(caption_indices, entities_encoded,
                            word_embedding, int(pad_token), caption_masks)
    res = run_bass_kernel_spmd(
        nc,
        in_maps,
        list(range(N_CORES)),
        trace=bool(os.environ.get("CAPEMB_TRACE")),
    )
    LAST_RESULTS = res
    out = np.empty((B, L, D), dtype=np.float32)
    for i in range(N_CORES):
        toks = np.transpose(res.results[i]["out"], (1, 0, 2)).reshape(
            TOK_PAD, D
        )[:TOK]
        out[i * B_LOC : (i + 1) * B_LOC] = toks.reshape(B_LOC, L, D)
    return out
